# revision 50
# baseline (speedup 1.0000x reference)
"""NT-Xent / SimCLR contrastive loss on 8 Trainium2 NeuronCores (v6.4).

Math (matches the jax reference):
    z = l2_normalize(concat([emb_i, emb_j]))          # [2B, D] unit rows
    sim = z @ z.T                                     # cosine similarities
    denom_r = sum_{j != r} exp(sim_rj / T)
    pos_r   = z_r . z_{(r+B) mod 2B}                  # the positive pair
    loss = mean_r( log(denom_r) - pos_r / T )

Sharding: the 2B=8192 rows are data-parallel over 8 cores. Each core
receives the representation matrix ROTATED by -c*1024 rows, so its slab
is always local rows 0..1023 (one SPMD program for all cores). By the
symmetry of exp(sim/T) each core computes only the 5120 columns
c*1024..c*1024+5119; the three missing 1024-column blocks of each
row-sum are column sums computed by cores c+5..c+7 and are exchanged
through the host, which assembles denominators and the scalar loss.

v6 layout: similarity blocks are computed COLUMN-MAJOR — the matmul
output partition dim is the column index. Then:
  - lhsT is the raw (host-transposed, fp8) representation tile: no
    on-device transposes and no column normalization pass at all;
  - rhs is the core's own 1024 rows, normalized once (2 chunks) via a
    PSUM scale plane (ones-matmul broadcast of transposed scales);
  - the missing column scale is applied INSIDE the exp: ACT's
    activation scale is a per-partition AP = 2/T * rsqrt(ssq) of the
    column block, fused with the temperature;
  - the exp's fused accumulator yields the per-column sums (the
    host-exchanged partial denominators) for free;
  - local row-sums are chained ones-DoubleRow matmuls over the fp8 exp
    tiles (deferred one pair behind the exp stream), accumulating across
    all 40 column tiles in 2 PSUM banks;
  - positives = diagonal of the cols-4096..5119 blocks, pulled from the
    raw-sim PSUM with an identity-masked multiply-accumulate, then
    column-scaled (folding 2/T) on DVE;
  - sum-of-squares on DVE (square+row-accumulate), rsqrt via integer
    seed + 1 Newton step on DVE (ACT stays exp-only: one table set);
  - warmups: a dummy exp hoists the ACT table load to ~3us, and dummy
    PE transposes hold the tensor-engine p-state hot through the ramp;
    group 0 runs as two independent half-row pipelines and chunk 0 as
    per-(coltile, half) tiles so exps start on the first zloc half.
Outputs per core: column sums [128,24] (chunks 2..7, the only ones
peers import), local row sums [2,512], positives [128,8] (pre-scaled);
the host subtracts e^2, takes logs and reduces.
"""

import numpy as np
from contextlib import ExitStack

import ml_dtypes
import concourse.bass as bass
import concourse.tile as tile
from concourse import bacc, mybir
from concourse._compat import with_exitstack
from concourse.bass_utils import run_bass_kernel_spmd

B = 4096
D = 256
R = 2 * B
N_CORES = 8
SLAB = R // N_CORES          # 1024 rows per core
INV_T = 2.0
E2 = float(np.exp(2.0))

NCOL = 5 * SLAB              # 5120 columns computed per core
NG = 5                       # load groups of 1024 rows/cols
NJ = 10                      # 512-column chunks
M_TILES = SLAB // 128        # 8

F32 = mybir.dt.float32
BF16 = mybir.dt.bfloat16
FP8 = mybir.dt.float8e4
NP_FP8 = ml_dtypes.float8_e4m3fn


@with_exitstack
def _loss_kernel(ctx: ExitStack, tc: "tile.TileContext", rs_ap: bass.AP,
                 pos_ap: bass.AP, dexp_ap: bass.AP, reps_ap: bass.AP,
                 repst8_ap: bass.AP, ones_ap: bass.AP, ident32_ap: bass.AP,
                 sel_ap: bass.AP):
    nc = tc.nc
    mult = mybir.AluOpType.mult
    add = mybir.AluOpType.add
    bypass = mybir.AluOpType.bypass
    subtract = mybir.AluOpType.subtract
    lsr = mybir.AluOpType.logical_shift_right
    Exp = mybir.ActivationFunctionType.Exp
    DR = mybir.MatmulPerfMode.DoubleRow
    I32 = mybir.dt.int32

    xpool = ctx.enter_context(tc.tile_pool(name="x", bufs=NG))
    xtpool = ctx.enter_context(tc.tile_pool(name="xt8", bufs=2))
    stats = ctx.enter_context(tc.tile_pool(name="stats", bufs=2))
    scales = ctx.enter_context(tc.tile_pool(name="scales", bufs=NG))
    sctpool = ctx.enter_context(tc.tile_pool(name="sct", bufs=2))
    prodpool = ctx.enter_context(tc.tile_pool(name="prod", bufs=2))
    zpool = ctx.enter_context(tc.tile_pool(name="zloc", bufs=1))
    cpool = ctx.enter_context(tc.tile_pool(name="const", bufs=1))
    epool = ctx.enter_context(tc.tile_pool(name="esc", bufs=20))
    accpool = ctx.enter_context(tc.tile_pool(name="acc", bufs=1))
    fpool = ctx.enter_context(tc.tile_pool(name="final", bufs=1))

    # PSUM: 2x2-bank matmul ping-pong + 2 banks rowsum + 2 banks planes
    mmpool = ctx.enter_context(tc.tile_pool(name="mm", bufs=3, space="PSUM"))
    rspool = ctx.enter_context(tc.tile_pool(name="rs", bufs=2, space="PSUM"))

    # ---- loads: all via gpsimd SWDGE (striped over the 16 queues) -------
    xg0 = [xpool.tile([128, 4, D], F32, tag="x0h", name=f"x0_{hh}")
           for hh in range(2)]
    xg = [None] + [xpool.tile([128, 8, D], F32, tag="x", name=f"x{g}")
                   for g in range(1, NG)]
    xt8 = [xtpool.tile([128, 5, 2, 512], FP8, tag="xt8", name=f"xt8_{h}")
           for h in range(2)]
    def load_xg(g):
        nc.gpsimd.dma_start(
            xg[g][:],
            reps_ap[g * 1024:(g + 1) * 1024, :].rearrange(
                "(t p) d -> p t d", p=128))

    def load_xt8(h):
        nc.gpsimd.dma_start(
            xt8[h][:],
            repst8_ap[5 * h:5 * (h + 1)].rearrange("j p k c -> p j k c"))

    for hh in range(2):
        nc.gpsimd.dma_start(
            xg0[hh][:],
            reps_ap[hh * 512:(hh + 1) * 512, :].rearrange(
                "(t p) d -> p t d", p=128))
    load_xt8(0)
    load_xt8(1)
    load_xg(1)
    load_xg(2)
    load_xg(3)
    load_xg(4)

    ones = cpool.tile([128, 2, 16], FP8, tag="ones")
    nc.sync.dma_start(ones[:], ones_ap[:])
    ident32 = cpool.tile([128, 128], F32, tag="ident32")
    nc.sync.dma_start(ident32[:], ident32_ap[:])
    sel = cpool.tile([8, 8, 128], BF16, tag="sel")
    nc.sync.dma_start(sel[:], sel_ap[:])
    ones8 = cpool.tile([8, 128], BF16, tag="ones8")
    nc.vector.memset(ones8[:], 1.0)
    magic = cpool.tile([128, 1], I32, tag="magic")
    nc.vector.memset(magic[:], 0x5F3759DF)
    warm = cpool.tile([128, 1], F32, tag="warm")
    warm2 = cpool.tile([128, 1], F32, tag="warm2")
    nc.vector.memset(warm[:], 0.0)
    nc.scalar.activation(warm2[:], warm[:], Exp)
    # keep the PE busy from ~5us so its p-state is hot for the first real
    # transpose/matmuls (cold PE runs at 0.65-1.2 GHz vs 2.4 sustained)
    for w in range(36):
        wt = mmpool.tile([128, 2, 512], F32, tag="mm", name=f"pewarm{w}")
        nc.tensor.transpose(wt[:, 0, 0:128], ident32[:], ident32[:])

    # ---- per-group scales: ssq -> rsqrt -> 2/T * sc ---------------------
    sc2g = [None]

    def emit_prep(g):
        ssq = stats.tile([128, 8], F32, tag="ssq", name=f"ssq{g}")
        for t in range(8):
            junk = prodpool.tile([128, D], F32, tag="prod", name=f"sq{g}_{t}")
            nc.vector.scalar_tensor_tensor(
                out=junk[:], in0=xg[g][:, t, :], scalar=1.0,
                in1=xg[g][:, t, :], op0=mult, op1=mult,
                accum_out=ssq[:, t:t + 1],
            )
        half = stats.tile([128, 8], I32, tag="half", name=f"half{g}")
        nc.vector.tensor_scalar(
            out=half[:], in0=ssq[:].bitcast(I32), scalar1=1, scalar2=None,
            op0=lsr,
        )
        y0 = stats.tile([128, 8], F32, tag="y0", name=f"y0{g}")
        nc.vector.scalar_tensor_tensor(
            out=y0[:].bitcast(I32), in0=magic[:].broadcast_to([128, 8]),
            scalar=0, in1=half[:], op0=bypass, op1=subtract,
        )
        yy = stats.tile([128, 8], F32, tag="yy", name=f"yy{g}")
        hh = stats.tile([128, 8], F32, tag="hh", name=f"hh{g}")
        sc = scales.tile([128, 8], F32, tag="scale", name=f"sc{g}")
        sc2 = scales.tile([128, 8], F32, tag="scale2", name=f"sc2_{g}")
        nc.vector.tensor_tensor(out=yy[:], in0=y0[:], in1=y0[:], op=mult)
        nc.vector.scalar_tensor_tensor(
            out=hh[:], in0=ssq[:], scalar=-0.5, in1=yy[:], op0=mult, op1=mult)
        nc.vector.scalar_tensor_tensor(
            out=sc[:], in0=hh[:], scalar=1.5, in1=y0[:], op0=add, op1=mult)
        nc.vector.tensor_scalar(
            out=sc2[:], in0=sc[:], scalar1=INV_T, scalar2=None, op0=mult)
        sc2g.append(sc2)
        return sc

    # ---- local rows 0..1023: two independent half-row pipelines ---------
    def emit_prep0_half(hh):
        ssq = stats.tile([128, 4], F32, tag="ssqh", name=f"ssqh{hh}")
        for t in range(4):
            junk = prodpool.tile([128, D], F32, tag="prod",
                                 name=f"sq0_{hh}_{t}")
            nc.vector.scalar_tensor_tensor(
                out=junk[:], in0=xg0[hh][:, t, :], scalar=1.0,
                in1=xg0[hh][:, t, :], op0=mult, op1=mult,
                accum_out=ssq[:, t:t + 1],
            )
        half = stats.tile([128, 4], I32, tag="halfh", name=f"halfh{hh}")
        nc.vector.tensor_scalar(
            out=half[:], in0=ssq[:].bitcast(I32), scalar1=1, scalar2=None,
            op0=lsr,
        )
        y0 = stats.tile([128, 4], F32, tag="y0h", name=f"y0h{hh}")
        nc.vector.scalar_tensor_tensor(
            out=y0[:].bitcast(I32), in0=magic[:].broadcast_to([128, 4]),
            scalar=0, in1=half[:], op0=bypass, op1=subtract,
        )
        yy = stats.tile([128, 4], F32, tag="yyh", name=f"yyh{hh}")
        hht = stats.tile([128, 4], F32, tag="hhh", name=f"hhh{hh}")
        sc = scales.tile([128, 4], F32, tag="scaleh", name=f"sch{hh}")
        sc2 = scales.tile([128, 4], F32, tag="scale2h", name=f"sc2h{hh}")
        nc.vector.tensor_tensor(out=yy[:], in0=y0[:], in1=y0[:], op=mult)
        nc.vector.scalar_tensor_tensor(
            out=hht[:], in0=ssq[:], scalar=-0.5, in1=yy[:], op0=mult,
            op1=mult)
        nc.vector.scalar_tensor_tensor(
            out=sc[:], in0=hht[:], scalar=1.5, in1=y0[:], op0=add, op1=mult)
        nc.vector.tensor_scalar(
            out=sc2[:], in0=sc[:], scalar1=INV_T, scalar2=None, op0=mult)
        return sc, sc2

    zloc = zpool.tile([128, 2, 1024], FP8, tag="zloc")
    sc2g0 = []
    bds = []
    for hh in range(2):
        scH, sc2H = emit_prep0_half(hh)
        sc2g0.append(sc2H)
        ppt = mmpool.tile([128, 2, 512], F32, tag="mm", name=f"scT0{hh}")
        nc.tensor.transpose(ppt[0:4, 0, 0:128], scH[:], ident32[:])
        bd = sctpool.tile([4, 4, 128], BF16, tag="bd", name=f"bd{hh}")
        nc.vector.tensor_tensor(
            out=bd[:],
            in0=ppt[0:4, 0, 0:128].unsqueeze(1).broadcast_to([4, 4, 128]),
            in1=sel[0:4, 0:4, :], op=mult)
        bds.append(bd)
    pls = []
    for hh in range(2):
        pl = mmpool.tile([128, 2, 512], F32, tag="mm", name=f"pl{hh}")
        nc.tensor.matmul(
            pl[:, 0, :], lhsT=ones8[0:4, :],
            rhs=bds[hh][:].rearrange("k t c -> k (t c)"))
        pls.append(pl)
    for hh in range(2):
        nc.vector.tensor_tensor(
            out=zloc[:, :, hh * 512:(hh + 1) * 512],
            in0=xt8[0][:, hh, :, :],
            in1=pls[hh][:, 0, :].unsqueeze(1).broadcast_to([128, 2, 512]),
            op=mult,
        )

    emit_prep(1)

    # ---- main stream: per column tile (j, i): 2 matmuls + exp -----------
    # pt[p=col, h=row half, c] = sim_raw[col = j*512+i*128+p, row = h*512+c]
    # exp scale AP = (2/T) * rsqrt(ssq_col) applied per partition.
    dexp = accpool.tile([128, 24], F32, tag="dexp")
    posneg = accpool.tile([128, M_TILES], F32, tag="posneg")
    posf = accpool.tile([128, M_TILES], F32, tag="posf")
    rs = [rspool.tile([16, 512], F32, tag="rs", name=f"rs{h}")
          for h in range(2)]
    esc_live = {}

    pending_links = []

    def flush_links(last=False):
        # emit the row-sum chain links for the previously finished esc
        # pair: deferring by one pair keeps PE from gating on its exp
        while pending_links:
            escp, first = pending_links.pop(0)
            for h in range(2):
                nc.tensor.matmul(
                    rs[h][:], lhsT=ones[:, :, :], rhs=escp[:, :, h, :],
                    perf_mode=DR, start=first, stop=last,
                )

    def emit_coltile(j, i):
        pt = mmpool.tile([128, 2, 512], F32, tag="mm", name=f"pt{j}_{i}")
        lhsT = xt8[j // 5][:, j % 5, :, i * 128:(i + 1) * 128]
        for h in range(2):
            nc.tensor.matmul(
                pt[:, h, :], lhsT=lhsT,
                rhs=zloc[:, :, h * 512:(h + 1) * 512], perf_mode=DR,
            )
        flush_links()
        if i % 2 == 0:
            esc_live[0] = epool.tile([128, 2, 2, 512], FP8, tag="esc",
                                     name=f"esc{j}_{i // 2}")
        esc = esc_live[0]
        # only cols 1024..4095 (chunks 2..7) are imported by peer cores
        if 2 <= j < 8:
            idx = 4 * (j - 2) + i
            acc = dexp[:, idx:idx + 1]
        else:
            acc = None
        if j < 2:
            scale = sc2g0[j][:, i:i + 1]
        else:
            scale = sc2g[j // 2][:, 4 * (j % 2) + i:4 * (j % 2) + i + 1]
        nc.scalar.activation(
            esc[:, i % 2, :, :], pt[:], Exp, scale=scale,
            accum_out=acc,
        )
        if j >= 8:
            # positives: diag of the cols-4096..5119 block; row half h=j-8
            junk = prodpool.tile([128, 128], F32, tag="posj",
                                 name=f"posj{j}_{i}", bufs=2)
            nc.vector.scalar_tensor_tensor(
                out=junk[:], in0=pt[:, j - 8, i * 128:(i + 1) * 128],
                scalar=1.0, in1=ident32[:], op0=mult, op1=mult,
                accum_out=posneg[:, 4 * (j - 8) + i:4 * (j - 8) + i + 1],
            )
        if i % 2 == 1:
            pending_links.append((esc, j == 0 and i == 1))

    # chunk 0: per-(coltile, half) tiles so the exp stream starts as soon
    # as zloc half 0 exists (no accum, no positives in this chunk)
    esc0 = {}
    for i in (0, 2):
        esc0[i // 2] = epool.tile([128, 2, 2, 512], FP8, tag="esc",
                                  name=f"esc0_{i // 2}")
    for h in range(2):
        for i in range(4):
            pt = mmpool.tile([128, 2, 512], F32, tag="mm",
                             name=f"p0_{h}_{i}")
            nc.tensor.matmul(
                pt[:, 0, :],
                lhsT=xt8[0][:, 0, :, i * 128:(i + 1) * 128],
                rhs=zloc[:, :, h * 512:(h + 1) * 512], perf_mode=DR,
            )
            nc.scalar.activation(
                esc0[i // 2][:, i % 2, h, :], pt[:, 0, :], Exp,
                scale=sc2g0[0][:, i:i + 1],
            )
            if h == 1 and i % 2 == 1:
                pending_links.append((esc0[i // 2], i == 1))

    for j in range(1, NJ):
        if j % 2 == 0 and 0 < j < 8:
            emit_prep(j // 2 + 1)
        for i in range(4):
            emit_coltile(j, i)
        if j == 7:
            nc.sync.dma_start(dexp_ap[:], dexp[:])
        if j == 8:
            nc.vector.tensor_tensor(
                out=posf[:, 0:4], in0=posneg[:, 0:4],
                in1=sc2g[4][:, 0:4], op=mult)
            nc.sync.dma_start(pos_ap[:, 0:4], posf[:, 0:4])

    flush_links(last=True)
    nc.vector.tensor_tensor(
        out=posf[:, 4:8], in0=posneg[:, 4:8], in1=sc2g[4][:, 4:8], op=mult)
    nc.sync.dma_start(pos_ap[:, 4:8], posf[:, 4:8])
    rsb = fpool.tile([128, 2, 512], F32, tag="rsb")
    nc.scalar.copy(rsb[0:1, 0, :], rs[0][0:1, :])
    nc.vector.tensor_copy(rsb[0:1, 1, :], rs[1][0:1, :])
    nc.sync.dma_start(rs_ap[:], rsb[0:1, :, :])


_CACHE = {}


def _get_compiled():
    if "nc" not in _CACHE:
        nc = bacc.Bacc("TRN2", target_bir_lowering=False, debug=False)
        reps_in = nc.dram_tensor("reps", [NCOL, D], F32, kind="ExternalInput")
        repst8_in = nc.dram_tensor("repsT8", [NJ, 128, 2, 512], FP8,
                                   kind="ExternalInput")
        ones_t = nc.inline_tensor(np.ones((128, 2, 16), dtype=NP_FP8),
                                  name="ones")
        ident32_t = nc.inline_tensor(np.eye(128, dtype=np.float32),
                                     name="ident32")
        sel_np = np.ascontiguousarray(np.repeat(
            np.eye(8, dtype=ml_dtypes.bfloat16)[:, :, None], 128, axis=2))
        sel_t = nc.inline_tensor(sel_np, name="sel")
        rs_out = nc.dram_tensor("rsum", [1, 2, 512], F32,
                                kind="ExternalOutput")
        pos_out = nc.dram_tensor("pos", [128, M_TILES], F32,
                                 kind="ExternalOutput")
        dexp_out = nc.dram_tensor("dexp", [128, 24], F32,
                                  kind="ExternalOutput")
        with tile.TileContext(nc) as tc:
            _loss_kernel(tc, rs_out.ap(), pos_out.ap(), dexp_out.ap(),
                         reps_in.ap(), repst8_in.ap(), ones_t.ap(),
                         ident32_t.ap(), sel_t.ap())
        nc.compile()
        _CACHE["nc"] = nc
    return _CACHE["nc"]


def make_in_maps(emb_i: np.ndarray, emb_j: np.ndarray):
    reps = np.concatenate(
        [np.asarray(emb_i, dtype=np.float32),
         np.asarray(emb_j, dtype=np.float32)],
        axis=0,
    )
    maps = []
    for c in range(N_CORES):
        rolled = np.ascontiguousarray(np.roll(reps, -c * SLAB, axis=0)[:NCOL])
        # rt8[j, p, k, c] = fp8(rolled[j*512 + c, k*128 + p])
        rt8 = np.ascontiguousarray(
            rolled.reshape(NJ, 512, 2, 128).transpose(0, 3, 2, 1)
        ).astype(NP_FP8)
        maps.append({"reps": rolled, "repsT8": rt8})
    return maps


def run_spmd(emb_i, emb_j, **kwargs):
    nc = _get_compiled()
    in_maps = make_in_maps(emb_i, emb_j)
    return run_bass_kernel_spmd(nc, in_maps, core_ids=list(range(N_CORES)),
                                **kwargs)


def combine(results) -> np.ndarray:
    """Host-side combine ("all-reduce"): per-row denominators = local row
    sums + the three missing blocks, which are the column-sum exports of
    cores c+5..c+7; subtract e^2 for the self column, take logs, add the
    positive terms, and reduce to the scalar loss."""
    rsl = [results[c]["rsum"].astype(np.float64).reshape(SLAB)
           for c in range(N_CORES)]
    pos = [results[c]["pos"].astype(np.float64).T.reshape(SLAB)
           for c in range(N_CORES)]
    # dexp[p, idx] = colsum of local col 1024 + idx*128 + p (chunks 2..7)
    dx = [results[c]["dexp"].astype(np.float64).T.reshape(24 * 128)
          for c in range(N_CORES)]
    total = 0.0
    for c in range(N_CORES):
        den = (rsl[c]
               + dx[(c + 5) % 8][2 * SLAB:3 * SLAB]
               + dx[(c + 6) % 8][1 * SLAB:2 * SLAB]
               + dx[(c + 7) % 8][0 * SLAB:1 * SLAB]
               - E2)
        total += float(np.sum(np.log(den) - pos[c]))
    return np.array(total / R, dtype=np.float32)


def kernel(emb_i: np.ndarray, emb_j: np.ndarray) -> np.ndarray:
    res = run_spmd(emb_i, emb_j)
    return combine(res.results)
